# revision 8
# baseline (speedup 1.0000x reference)
"""Expert-parallel MoE layer for Trainium2 (8 NeuronCores, one expert per core).

Host side (numpy): router logits, exact top-2 dispatch, p0 weights, and the
scatter-add combine. Device side (Bass/Tile, SPMD over 8 cores): the dense FFN
y = gelu(x @ W1[e] + b1[e]) @ W2[e] over the tokens routed to expert e,
computed with fp16 operands (fp32 PSUM accumulation).

v2 layout: tokens ride the MOVING dim of BOTH GEMMs, so the per-core token
capacity is the exact max expert load (no 128-row padding), and GEMM2 consumes
h^T [F, tokens] directly, producing y^T [H, tokens] written to DRAM exactly
once (no DRAM read-modify-write accumulation like v1's quartered scheme).
Full W1 and W2 stay SBUF-resident in fp16 (128 KB/partition of the 224 KB).

Per token group (TT=512, tail-sized last group):
  GEMM1: psum[f128, t] = sum_k w1[k, f128]^T x^T[k, t]   (8 k-chunks over H)
  gelu+bias -> h[f128-chunk, t] fp16                     (32 f-chunks)
  GEMM2: psum[h'128, t] = sum_k2 w2[k2, h'128]^T h[k2, t] (32 k2-chunks over F)
  copy -> y^T stage -> single DMA store per group
"""

import numpy as np

B, S, H, E, F = 4, 2048, 1024, 8, 4096
T = B * S
P = 128
TT = 512            # token group size (moving free dim of both GEMMs)
KH = H // P         # 8  k-chunks over H  (GEMM1 contraction)
KF = F // P         # 32 k-chunks over F  (GEMM2 contraction)
NH = H // P         # 8  output h'-chunks of GEMM2

_cache = {}


def _spill_waits(nc, mybir, max_waits=1):
    """walrus CoreV2/V3 codegen rejects instructions with >1 semaphore wait
    ("Too many sync wait commands"). Move excess waits onto same-engine no-ops
    inserted right before the instruction (sequencers run in order, so this is
    equivalent)."""
    for fn in nc.m.functions:
        for blk in fn.blocks:
            out = []
            changed = False
            for inst in blk.instructions:
                si = getattr(inst, "sync_info", None)
                if si is not None and len(si.on_wait) > max_waits:
                    spill = si.on_wait[: len(si.on_wait) - max_waits]
                    keep = si.on_wait[len(si.on_wait) - max_waits:]
                    for w in spill:
                        nop = mybir.InstNoOp(
                            name=nc.get_next_instruction_name(),
                            engine=inst.engine,
                            ins=[],
                            outs=[],
                        )
                        nop.sync_info = mybir.SyncInfo(on_wait=[w], on_update=[])
                        out.append(nop)
                    inst.sync_info = mybir.SyncInfo(on_wait=keep, on_update=si.on_update)
                    changed = True
                out.append(inst)
            if changed:
                blk.instructions = out


def _build(cap):
    import concourse.bass as bass
    import concourse.mybir as mybir
    from concourse import tile

    F32 = mybir.dt.float32
    SDT = mybir.dt.float16
    GELU = mybir.ActivationFunctionType.Gelu_apprx_tanh

    nc = bass.Bass()
    xt = nc.declare_dram_parameter("xt", [H, cap], SDT, isOutput=False)
    w1 = nc.declare_dram_parameter("w1", [H, F], SDT, isOutput=False)
    w2 = nc.declare_dram_parameter("w2", [F, H], SDT, isOutput=False)
    b1s = nc.declare_dram_parameter("b1s", [P, KF], F32, isOutput=False)
    yt = nc.declare_dram_parameter("yt", [H, cap], F32, isOutput=True)

    # token groups: full TT groups first, tail last (w1/w2 stream during g0)
    groups = []
    o = 0
    while o < cap:
        tt = min(TT, cap - o)
        groups.append((o, tt))
        o += tt

    xsrc = xt.rearrange("(c p) t -> p c t", p=P)
    w1src = w1.rearrange("(c p) f -> p c f", p=P)
    w2src = w2.rearrange("(c p) h -> p c h", p=P)
    ydst = yt.rearrange("(c p) t -> p c t", p=P)

    with tile.TileContext(nc) as tc:
        with (
            tc.tile_pool(name="w1p", bufs=1) as w1p,
            tc.tile_pool(name="w2p", bufs=1) as w2p,
            tc.tile_pool(name="xp", bufs=1) as xp,
            tc.tile_pool(name="hp", bufs=1) as hp,
            tc.tile_pool(name="yp", bufs=4) as yp,
            tc.tile_pool(name="cst", bufs=1) as cst,
            tc.tile_pool(name="ps1", bufs=4, space="PSUM") as ps1,
            tc.tile_pool(name="ps2", bufs=4, space="PSUM") as ps2,
        ):
            # Startup DMA orchestration. Two constraints drive the layout:
            # (1) Tile hands DMA-completion semaphore lanes out of a shared
            # pool of 8 -- a huge DMA parks a lane for its whole transfer and
            # stalls later DMAs that need the lane back; (2) each dma_start
            # costs ~0.7us of the ISSUING engine's sequencer, and the scalar
            # engine must be free to run gelu as soon as GEMM1 psum group 0
            # lands. So: scalar issues only the 4 x loads (k-halves of group
            # 0 first, so the first matmul unblocks after 512KB), sync issues
            # bias + w1 + w2 in consumption order -- fs-pair chunks for w1's
            # first quarter (fs group 0 unblocks after 512KB), quarters after,
            # then w2 (first needed by GEMM2(g0) ~55us in).
            w1r = w1p.tile([P, KH, F], SDT, tag="w1r")
            x_all = xp.tile([P, KH, cap], SDT, tag="x")
            b1t = cst.tile([P, KF], F32)
            w2r = w2p.tile([P, KF, H], SDT, tag="w2r")

            nc.scalar.dma_start(x_all[:, :2, :TT], xsrc[:, :2, :TT])
            nc.scalar.dma_start(x_all[:, 2:5, :TT], xsrc[:, 2:5, :TT])
            nc.scalar.dma_start(x_all[:, 5:, :TT], xsrc[:, 5:, :TT])
            nc.sync.dma_start(b1t[:], b1s[:])
            FQ = F // 4
            nc.sync.dma_start(w1r[:, :, 0:P], w1src[:, :, 0:P])
            for fp in range(1, 4):  # rest of quarter 0 in fs-pair-ish chunks
                lo = P + (fp - 1) * 320
                hi = min(P + fp * 320, FQ)
                nc.sync.dma_start(w1r[:, :, lo:hi], w1src[:, :, lo:hi])
            nc.scalar.dma_start(
                w1r[:, :, FQ:2 * FQ], w1src[:, :, FQ:2 * FQ]
            )
            for q in range(2, 4):
                nc.sync.dma_start(
                    w1r[:, :, q * FQ:(q + 1) * FQ], w1src[:, :, q * FQ:(q + 1) * FQ]
                )
            for kc in range(0, KF, 8):
                nc.sync.dma_start(
                    w2r[:, kc:kc + 8, :], w2src[:, kc:kc + 8, :]
                )
            rest = (cap - TT + 1) // 2
            nc.scalar.dma_start(
                x_all[:, :, TT:TT + rest], xsrc[:, :, TT:TT + rest]
            )
            nc.scalar.dma_start(
                x_all[:, :, TT + rest:], xsrc[:, :, TT + rest:]
            )

            # PE clock warm-up: the HAM clock gate holds the PE at 1.2 GHz
            # until it has seen ~3.4us of sustained matmul activity, and the
            # first real matmul can't start until w1/x land (~13us). Run
            # dependency-free dummy matmuls (one stationary load, moving 512)
            # through that window so the real GEMM starts at 2.4 GHz.
            warm = cst.tile([P, 640], SDT, tag="warm")
            nc.vector.memset(warm[:], 0)
            for _ in range(52):
                pw = ps2.tile([P, TT], F32, tag="pt2")
                nc.tensor.matmul(
                    pw[:], warm[:, 0:P], warm[:, P:P + TT], start=True, stop=True
                )

            h = hp.tile([P, KF, TT], SDT, tag="h")
            for gi, (t0, tt) in enumerate(groups):
                # GEMM1: h^T[f, t] = gelu(sum_k W1[k, f] * x^T[k, t] + b1[f])
                for fs in range(KF):
                    pt = ps1.tile([P, TT], F32, tag="pt1")
                    for k in range(KH):
                        nc.tensor.matmul(
                            pt[:, :tt],
                            w1r[:, k, fs * P:(fs + 1) * P],
                            x_all[:, k, t0:t0 + tt],
                            start=(k == 0),
                            stop=(k == KH - 1),
                        )
                    nc.scalar.activation(
                        h[:, fs, :tt], pt[:, :tt], GELU, bias=b1t[:, fs:fs + 1]
                    )
                # GEMM2: y^T[h', t] = sum_k2 W2[k2, h'] * h^T[k2, t]
                # per-h'-chunk staging+store so the tail drains during the
                # last copies (and the stage stays at 4x2KB of SBUF)
                for n in range(NH):
                    pt2 = ps2.tile([P, TT], F32, tag="pt2")
                    for k2 in range(KF):
                        nc.tensor.matmul(
                            pt2[:, :tt],
                            w2r[:, k2, n * P:(n + 1) * P],
                            h[:, k2, :tt],
                            start=(k2 == 0),
                            stop=(k2 == KF - 1),
                        )
                    stage = yp.tile([P, TT], F32, tag="stage")
                    nc.vector.tensor_copy(stage[:, :tt], pt2[:, :tt])
                    nc.sync.dma_start(
                        ydst[:, n, t0:t0 + tt], stage[:, :tt]
                    )

    import concourse.mybir as mybir_mod

    _spill_waits(nc, mybir_mod)
    return nc


def _route(x2d, Wr, br):
    """Top-2 routing, bit-matching the reference's decisions.

    Softmax is monotonic, so top-2-of-probs == top-2-of-logits, and the
    normalized top-1 weight p0 = p1/(p1+p2) == sigmoid(l1-l2) exactly (the
    softmax denominator cancels). Ordering ties are broken by lower index,
    same as jax.lax.top_k."""
    logits = x2d @ np.asarray(Wr, np.float32) + np.asarray(br, np.float32)
    order = np.argsort(-logits, axis=-1, kind="stable")
    i1 = order[:, 0].astype(np.int64)
    i2 = order[:, 1].astype(np.int64)
    r = np.arange(logits.shape[0])
    l1 = logits[r, i1].astype(np.float64)
    l2 = logits[r, i2].astype(np.float64)
    p0 = 1.0 / (1.0 + np.exp(l2 - l1))
    return i1, i2, p0.astype(np.float32)


def _prepare(x, Wr, br, W1, b1, W2, b2):
    """Route on host, build per-core input maps and the (cached) Bass program."""
    x2d = np.ascontiguousarray(np.asarray(x, np.float32).reshape(T, H))
    W1 = np.asarray(W1, np.float32)
    b1 = np.asarray(b1, np.float32)
    W2 = np.asarray(W2, np.float32)

    i1, i2, p0 = _route(x2d, Wr, br)
    idxs = [np.flatnonzero((i1 == e) | (i2 == e)) for e in range(E)]
    cap = max(len(ix) for ix in idxs)

    if cap not in _cache:
        _cache[cap] = _build(cap)
    nc = _cache[cap]

    xT = np.ascontiguousarray(x2d.T)  # [H, T]
    in_maps = []
    for e in range(E):
        ix = idxs[e]
        xte = np.zeros((H, cap), np.float16)
        xte[:, : len(ix)] = xT[:, ix]
        b1se = np.ascontiguousarray(b1[e].reshape(KF, P).T)
        in_maps.append(
            {
                "xt": xte,
                "w1": np.ascontiguousarray(W1[e]).astype(np.float16),
                "w2": np.ascontiguousarray(W2[e]).astype(np.float16),
                "b1s": b1se,
            }
        )
    return nc, in_maps, idxs, p0


def _combine(res, idxs, p0, b2):
    b2 = np.asarray(b2, np.float32)
    out = np.zeros((T, H), np.float32)
    for e in range(E):
        ix = idxs[e]
        ye = res.results[e]["yt"][:, : len(ix)].T  # [n_e, H]
        out[ix] += p0[ix, None] * (ye + b2[e][None, :])
    return out.reshape(B, S, H)


def kernel(x, Wr, br, W1, b1, W2, b2):
    from concourse.bass_utils import run_bass_kernel_spmd

    nc, in_maps, idxs, p0 = _prepare(x, Wr, br, W1, b1, W2, b2)
    try:
        res = run_bass_kernel_spmd(nc, in_maps, list(range(E)))
    except Exception:
        import time as _time

        _time.sleep(10)
        res = run_bass_kernel_spmd(nc, in_maps, list(range(E)))
    return _combine(res, idxs, p0, b2)
